# revision 22
# baseline (speedup 1.0000x reference)
"""Local (sliding-window w=2) attention, B=4 S=2048 H=1024, on 8 trn2 cores.

Strategy: sequence-parallel. Each core owns half of one batch's sequence
(1024 tokens) plus a 2-token halo on each side (ext = 1028 tokens).

Per core:
  Q^T/K^T projections feature-major [H, T] in *fp8e4m3 with
  perf_mode=DoubleRow* (K=256 per matmul, ~1.7x the fp16 rate; x scaled
  by 16 and W by 64 into the fp8 sweet range, biases pre-scaled by 1024
  on host, the 1/1024^2 descale folded into the softmax exp scale;
  measured end-to-end rel err 1.55e-2 vs the 2e-2 gate).  V stays fp16
  (its quantization error would land directly in the output).  Then 9
  q-blocks of 123 queries: band scores (window 127) off the scaled fp16
  Q^T/K^T, masked softmax (ACT exp + fused row-sum), P transpose on PE,
  P@V (bias via extra V row + ones column), fp16 output.

Lessons baked in from traces of previous versions:
  - HAM warm-up needs *real* matmuls (transpose-mode doesn't count).
  - every input chunk is a separate DRAM tensor, contiguous per
    partition (2-8KB descriptors); sub-slicing one big tensor made
    512-1KB descriptors and dropped input BW from ~310 to ~210 GB/s.
  - inputs stream on both HWDGE rings in first-use order: x8/wq8 (Q),
    x8/wk8 (K), xv/wv (V).
  - SBUF->HBM writes: both HWDGE rings funnel through the same 3 SDMA
    engines (~78 GB/s combined), SWDGE (gpsimd) spreads over ~6 others;
    output rotates sync/scalar/gpsimd, small last block on sync so the
    end-of-kernel drain is short, gpsimd's last block is b=6 (SWDGE
    teardown is slow).
  - PSUM: pproj 2 banks, patt 2, pout 3x1 bank, ptp 1 = 8.
"""

import os
import sys

sys.path.insert(0, "/opt/trn_rl_repo")

import ml_dtypes
import numpy as np

import concourse.bass as bass  # noqa: F401  (bass must import before tile)
import concourse.mybir as mybir
import concourse.tile as tile
from concourse import bacc
from concourse.bass_utils import run_bass_kernel_spmd

F32 = mybir.dt.float32
F16 = mybir.dt.float16
F8 = mybir.dt.float8e4

B, S, H = 4, 2048, 1024
WCTX = 2
NCORES = 8
SHARD = S // 2  # tokens per core
EXT = SHARD + 2 * WCTX  # 1028
EXTP = 1040  # fp8 x tile padded so the hc-axis stride is 16B-aligned
P = 128
QB = 123  # queries per attention block
WIN = QB + 2 * WCTX  # 127 = key window per block
NBLK = (SHARD + QB - 1) // QB  # 9
HC = H // P  # 8 feature chunks
HH = HC // 2
TH = 516  # token split of x between the two projection halves
SCALE = 1.0 / np.sqrt(np.float32(H))
XS, WS = 16.0, 64.0  # fp8 input scales (x, weights)
QKS = XS * WS  # Q/K are computed scaled by 1024

_prog_cache = {}


def _build_program():
    nc = bacc.Bacc("TRN2", target_bir_lowering=False, debug=False)

    def din(name, shape, dt):
        return nc.dram_tensor(name, shape, dt, kind="ExternalInput").ap()

    # fp8 x/wq/wk chunks and fp16 xv/wv chunks, one DRAM tensor per
    # (ring, piece) so every transfer is contiguous per partition
    x8a = [din(f"x8a{i}", [P, HH * TH], F8) for i in range(2)]
    x8b = [din(f"x8b{i}", [P, HH * (EXT - TH)], F8) for i in range(2)]
    # weights quartered (jc-half x hc-half) so the first Q/K matmul group
    # only waits for half the weight bytes
    wq8 = [[din(f"wq8{j}{i}", [P, HH * 512], F8) for i in range(2)] for j in range(2)]
    wk8 = [[din(f"wk8{j}{i}", [P, HH * 512], F8) for i in range(2)] for j in range(2)]
    xv = [din(f"xv{i}", [P, HH * EXT], F16) for i in range(2)]
    wv = [din(f"wv{i}", [P, HH * H], F16) for i in range(2)]
    bq_d = din("bq_c", [P, HC], F32)
    bk_d = din("bk_c", [P, HC], F32)
    bv_d = din("bv_r", [P, H], F16)
    id_d = din("ident", [P, P], F16)
    mk_d = din("mask", [NBLK, QB, WIN], F32)
    out_d = nc.dram_tensor("out", [SHARD, H], F16, kind="ExternalOutput").ap()

    def r3(ap, n):
        return ap.rearrange("p (hc n) -> p hc n", hc=HH)

    mk_r = mk_d.rearrange("b q c -> q b c")

    with tile.TileContext(nc) as tc:
        with (
            tc.tile_pool(name="persist", bufs=1) as pers,
            tc.tile_pool(name="vpool", bufs=4) as vpool,
            tc.tile_pool(name="spool", bufs=4) as spool,
            tc.tile_pool(name="opool", bufs=4) as opool,
            tc.tile_pool(name="pproj", bufs=2, space="PSUM") as pproj,
            tc.tile_pool(name="patt", bufs=2, space="PSUM") as patt,
            tc.tile_pool(name="pout", bufs=3, space="PSUM") as pout,
            tc.tile_pool(name="ptp", bufs=1, space="PSUM") as ptp,
        ):
            # ---- HAM warm-up: real matmuls on a memset tile ----
            warm = pers.tile([P, 640], F16)
            with tc.high_priority():
                nc.vector.memset(warm[:], 0.25)
                for _ in range(13):
                    psw = pproj.tile([P, 512], F32, tag="proj")
                    nc.tensor.matmul(
                        psw[:], warm[:, :128], warm[:, 128:], start=True, stop=True
                    )

            # ---- inputs in first-use order across both HWDGE rings ----
            xt8 = pers.tile([P, HC, EXTP], F8)
            # weights laid out [P, jc-half, hc, 512] so each quarter-DMA is
            # contiguous per partition (2KB descriptors)
            wq8_sb = pers.tile([P, 2, HC, 512], F8)
            wk8_sb = pers.tile([P, 2, HC, 512], F8)
            xtv = pers.tile([P, HC, EXT], F16)
            wv_sb = pers.tile([P, HC, H], F16)
            rings = (nc.sync, nc.scalar)
            for i, ring in enumerate(rings):
                h0, h1 = i * HH, (i + 1) * HH
                ring.dma_start(xt8[:, h0:h1, :TH], r3(x8a[i], TH))
            for j in range(2):
                for i, ring in enumerate(rings):
                    h0, h1 = i * HH, (i + 1) * HH
                    ring.dma_start(wq8_sb[:, j, h0:h1, :], r3(wq8[j][i], 512))
            for i, ring in enumerate(rings):
                h0, h1 = i * HH, (i + 1) * HH
                ring.dma_start(xt8[:, h0:h1, TH:EXT], r3(x8b[i], EXT - TH))
            for j in range(2):
                for i, ring in enumerate(rings):
                    h0, h1 = i * HH, (i + 1) * HH
                    ring.dma_start(wk8_sb[:, j, h0:h1, :], r3(wk8[j][i], 512))
            for i, ring in enumerate(rings):
                h0, h1 = i * HH, (i + 1) * HH
                ring.dma_start(xtv[:, h0:h1, :], r3(xv[i], EXT))
            for i, ring in enumerate(rings):
                h0, h1 = i * HH, (i + 1) * HH
                ring.dma_start(wv_sb[:, h0:h1, :], r3(wv[i], H))

            # ---- small constants on the gpsimd (SWDGE) queue ----
            bqc = pers.tile([P, HC], F32)
            nc.gpsimd.dma_start(bqc[:], bq_d)
            bkc = pers.tile([P, HC], F32)
            nc.gpsimd.dma_start(bkc[:], bk_d)
            ident = pers.tile([P, P], F16)
            nc.gpsimd.dma_start(ident[:], id_d)
            maskt = pers.tile([QB, NBLK, WIN], F32)
            nc.gpsimd.dma_start(maskt[:], mk_r)
            bvr = pers.tile([P, H], F16)
            nc.gpsimd.dma_start(bvr[:], bv_d)

            DR = mybir.MatmulPerfMode.DoubleRow

            # ---- Q^T projection: owned tokens ext [2, 1026) ----
            qt_sb = pers.tile([P, HC, SHARD], F16)
            with nc.named_scope("qproj"):
                for t in range(2):
                    for jc in range(HC):
                        ps = pproj.tile([P, 512], F32, tag="proj")
                        for g in range(HH):
                            nc.tensor.matmul(
                                ps[:],
                                wq8_sb[
                                    :,
                                    jc // 4,
                                    2 * g : 2 * g + 2,
                                    (jc % 4) * P : (jc % 4 + 1) * P,
                                ],
                                xt8[:, 2 * g : 2 * g + 2, 2 + 512 * t : 2 + 512 * (t + 1)],
                                start=(g == 0),
                                stop=(g == HH - 1),
                                perf_mode=DR,
                            )
                        nc.vector.tensor_scalar_add(
                            qt_sb[:, jc, 512 * t : 512 * (t + 1)],
                            ps[:],
                            bqc[:, jc : jc + 1],
                        )

            # ---- K^T projection: all ext tokens [0, 1028); the tiny halo
            # chunk is emitted later, interleaved with the first V block
            # (a burst of 64 N=4 matmuls looks idle to the HAM clock gate
            # and re-throttled the PE right at the attention transition) ----
            kt_sb = pers.tile([P, HC, EXT], F16)

            def emit_k_chunk(t0, t1, jcs):
                n = t1 - t0
                for jc in jcs:
                    ps = pproj.tile([P, 512], F32, tag="proj")
                    for g in range(HH):
                        nc.tensor.matmul(
                            ps[:, :n],
                            wk8_sb[
                                :,
                                jc // 4,
                                2 * g : 2 * g + 2,
                                (jc % 4) * P : (jc % 4 + 1) * P,
                            ],
                            xt8[:, 2 * g : 2 * g + 2, t0:t1],
                            start=(g == 0),
                            stop=(g == HH - 1),
                            perf_mode=DR,
                        )
                    nc.vector.tensor_scalar_add(
                        kt_sb[:, jc, t0:t1], ps[:, :n], bkc[:, jc : jc + 1]
                    )

            with nc.named_scope("kproj"):
                emit_k_chunk(0, 512, range(HC))
                emit_k_chunk(512, 1024, range(HC))

            # ---- attention blocks, software-pipelined by two stages ----
            def blk_geom(b):
                q0 = QB * b
                qb = min(QB, SHARD - q0)
                return q0, qb, qb + 2 * WCTX

            def emit_v_half(b, n, vb):
                q0, qb, w = blk_geom(b)
                psv = pproj.tile([P, 512], F32, tag="proj")
                for hc in range(HC):
                    nc.tensor.matmul(
                        psv[:w, :],
                        xtv[:, hc, q0 : q0 + w],
                        wv_sb[:, hc, 512 * n : 512 * (n + 1)],
                        start=(hc == 0),
                        stop=(hc == HC - 1),
                    )
                nc.scalar.copy(vb[:w, 512 * n : 512 * (n + 1)], psv[:w, :])

            def emit_v_bias(b, vb):
                q0, qb, w = blk_geom(b)
                nc.gpsimd.dma_start(vb[w : w + 1, :], bvr[w : w + 1, :])

            def emit_scores_softmax(b):
                q0, qb, w = blk_geom(b)
                pss = patt.tile([QB, WIN], F32, tag="ps")
                for jc in range(HC):
                    nc.tensor.matmul(
                        pss[:qb, :w],
                        qt_sb[:, jc, q0 : q0 + qb],
                        kt_sb[:, jc, q0 : q0 + w],
                        start=(jc == 0),
                        stop=(jc == HC - 1),
                    )
                sm = spool.tile([QB, WIN], F32, tag="sm")
                nc.vector.tensor_tensor(
                    sm[:qb, :w], pss[:qb, :w], maskt[:qb, b, :w], op=mybir.AluOpType.add
                )
                pexp = spool.tile([QB, WIN], F32, tag="pexp")
                rsum = spool.tile([QB, 1], F32, tag="rsum")
                nc.scalar.activation(
                    pexp[:qb, :w],
                    sm[:qb, :w],
                    mybir.ActivationFunctionType.Exp,
                    bias=0.0,
                    scale=float(SCALE / (QKS * QKS)),
                    accum_out=rsum[:qb],
                )
                rcp = spool.tile([QB, 1], F32, tag="rcp")
                nc.vector.reciprocal(rcp[:qb], rsum[:qb])
                pn = spool.tile([QB, WIN + 1], F16, tag="pn")
                nc.vector.tensor_scalar_mul(pn[:qb, :w], pexp[:qb, :w], rcp[:qb])
                nc.vector.memset(pn[:qb, w : w + 1], 1.0)
                return pn

            def emit_transpose(b, pn):
                q0, qb, w = blk_geom(b)
                pst = ptp.tile([WIN + 1, QB], F16, tag="pt")
                nc.tensor.transpose(pst[: w + 1, :qb], pn[:qb, : w + 1], ident[:qb, :qb])
                pts = spool.tile([WIN + 1, QB], F16, tag="pts")
                nc.scalar.copy(pts[: w + 1, :qb], pst[: w + 1, :qb])
                return pts

            OUTQ = {0: 0, 1: 1, 2: 2, 3: 0, 4: 1, 5: 2, 6: 0, 7: 1, 8: 0}

            def emit_pv_out(b, pts, vb):
                q0, qb, w = blk_geom(b)
                ob = opool.tile([QB, H], F16, tag="ob")
                for n in range(2):
                    pso = pout.tile([QB, 512], F32, tag="po")
                    nc.tensor.matmul(
                        pso[:qb, :],
                        pts[: w + 1, :qb],
                        vb[: w + 1, 512 * n : 512 * (n + 1)],
                        start=True,
                        stop=True,
                    )
                    eng = nc.vector.tensor_copy if n == 0 else nc.scalar.copy
                    eng(ob[:qb, 512 * n : 512 * (n + 1)], pso[:qb, :])
                dma_eng = (nc.sync, nc.scalar, nc.gpsimd)[OUTQ[b]]
                dma_eng.dma_start(out_d[q0 : q0 + qb, :], ob[:qb, :])

            with nc.named_scope("attn"):
                # prologue: blocks 0 and 1, with the tiny K halo chunk
                # sandwiched between dense V matmul bursts
                stage = []
                vb0 = vpool.tile([P, H], F16, tag="vblk")
                emit_v_half(0, 0, vb0)
                emit_k_chunk(1024, EXT, range(0, 4))
                emit_v_half(0, 1, vb0)
                emit_v_bias(0, vb0)
                emit_k_chunk(1024, EXT, range(4, HC))
                stage.append((vb0, emit_scores_softmax(0)))
                vb1 = vpool.tile([P, H], F16, tag="vblk")
                emit_v_half(1, 0, vb1)
                emit_v_half(1, 1, vb1)
                emit_v_bias(1, vb1)
                stage.append((vb1, emit_scores_softmax(1)))
                # per iteration: transpose+pts first so the pts copy heads the
                # ACT queue and is long done when PV issues after the V burst
                for b in range(NBLK):
                    vb, pn = stage[b]
                    pts = emit_transpose(b, pn)
                    vb2 = None
                    if b + 2 < NBLK:
                        vb2 = vpool.tile([P, H], F16, tag="vblk")
                        emit_v_half(b + 2, 0, vb2)
                        emit_v_half(b + 2, 1, vb2)
                        emit_v_bias(b + 2, vb2)
                    emit_pv_out(b, pts, vb)
                    if vb2 is not None:
                        stage.append((vb2, emit_scores_softmax(b + 2)))

    nc.compile()
    return nc


def _build_mask(h: int) -> np.ndarray:
    mask = np.full((NBLK, QB, WIN), -1e30, dtype=np.float32)
    r = np.arange(QB)[:, None]
    c = np.arange(WIN)[None, :]
    band = (c - r >= 0) & (c - r <= 2 * WCTX)
    for b in range(NBLK):
        q0 = QB * b
        qb = min(QB, SHARD - q0)
        gk = h * SHARD + q0 + c - WCTX  # global key token index
        valid = band & (gk >= 0) & (gk < S) & (r < qb) & (c < qb + 2 * WCTX)
        mask[b] = np.where(valid, np.float32(0.0), np.float32(-1e30))
    return mask


def _pmaj(w: np.ndarray, dtype) -> list[np.ndarray]:
    """[H, N] -> two ring-halves [P, HH*N], partition-contiguous."""
    n = w.shape[1]
    a = w.reshape(HC, P, n).transpose(1, 0, 2)  # [P, HC, n]
    return [
        np.ascontiguousarray(a[:, i * HH : (i + 1) * HH].reshape(P, HH * n)).astype(
            dtype
        )
        for i in range(2)
    ]


def kernel(sequence_output, Wq, bq, Wk, bk, Wv, bv):
    x = np.asarray(sequence_output, dtype=np.float32)
    Wq = np.asarray(Wq, dtype=np.float32)
    Wk = np.asarray(Wk, dtype=np.float32)
    Wv = np.asarray(Wv, dtype=np.float32)
    bq = np.asarray(bq, dtype=np.float32)
    bk = np.asarray(bk, dtype=np.float32)
    bv = np.asarray(bv, dtype=np.float32)

    if "nc" not in _prog_cache:
        _prog_cache["nc"] = _build_program()
    nc = _prog_cache["nc"]

    f8 = ml_dtypes.float8_e4m3fn
    wq8 = [_pmaj(Wq[:, 512 * j : 512 * (j + 1)] * WS, f8) for j in range(2)]
    wk8 = [_pmaj(Wk[:, 512 * j : 512 * (j + 1)] * WS, f8) for j in range(2)]
    wv_h = _pmaj(Wv, np.float16)
    bq_c = np.ascontiguousarray(bq.reshape(HC, P).T) * np.float32(QKS)
    bk_c = np.ascontiguousarray(bk.reshape(HC, P).T) * np.float32(QKS)
    bv_r = np.ascontiguousarray(np.broadcast_to(bv, (P, H))).astype(np.float16)
    ident = np.eye(P, dtype=np.float16)
    masks = [_build_mask(0), _build_mask(1)]

    # pad each sequence with WCTX zero rows on both ends, slice ext windows
    xp = np.zeros((B, S + 2 * WCTX, H), dtype=np.float32)
    xp[:, WCTX : WCTX + S] = x

    in_maps = []
    for c in range(NCORES):
        bidx, h = divmod(c, 2)
        ext = xp[bidx, h * SHARD : h * SHARD + EXT]  # [EXT, H]
        xT = np.ascontiguousarray(ext.T)  # [H, EXT]
        x8a = _pmaj(xT[:, :TH] * XS, f8)
        x8b = _pmaj(xT[:, TH:] * XS, f8)
        xv = _pmaj(xT, np.float16)
        im = {
            "bq_c": bq_c,
            "bk_c": bk_c,
            "bv_r": bv_r,
            "ident": ident,
            "mask": masks[h],
        }
        for i in range(2):
            im[f"x8a{i}"] = x8a[i]
            im[f"x8b{i}"] = x8b[i]
            im[f"xv{i}"] = xv[i]
            im[f"wv{i}"] = wv_h[i]
            for j in range(2):
                im[f"wq8{j}{i}"] = wq8[j][i]
                im[f"wk8{j}{i}"] = wk8[j][i]
        in_maps.append(im)

    trace = bool(int(os.environ.get("LK_TRACE", "0")))
    res = run_bass_kernel_spmd(
        nc,
        in_maps,
        core_ids=list(range(NCORES)),
        trace=trace,
        trace_cores=list(range(NCORES)) if trace else None,
    )
    _prog_cache["last_results"] = res

    out = np.empty((B, S, H), dtype=np.float32)
    for c in range(NCORES):
        bidx, h = divmod(c, 2)
        out[bidx, h * SHARD : (h + 1) * SHARD] = res.results[c]["out"].astype(
            np.float32
        )
    return out


# revision 24
# speedup vs baseline: 1.1279x; 1.1279x over previous
"""Local (sliding-window w=2) attention, B=4 S=2048 H=1024, on 8 trn2 cores.

Strategy: sequence-parallel. Each core owns half of one batch's sequence
(1024 tokens) plus a 2-token halo on each side (ext = 1028 tokens).

Per core:
  Q^T/K^T projections feature-major [H, T] in *fp8e4m3 with
  perf_mode=DoubleRow* (K=256 per matmul, ~1.7x the fp16 rate; x scaled
  by 16 and W by 64 into the fp8 sweet range, biases pre-scaled by 1024
  on host, the 1/1024^2 descale folded into the softmax exp scale;
  measured end-to-end rel err 1.55e-2 vs the 2e-2 gate).  V stays fp16
  (its quantization error would land directly in the output).  Then 9
  q-blocks of 123 queries: band scores (window 127) off the scaled fp16
  Q^T/K^T, masked softmax (ACT exp + fused row-sum), P transpose on PE,
  P@V (bias via extra V row + ones column), fp16 output.

Lessons baked in from traces of previous versions:
  - HAM warm-up needs *real* matmuls (transpose-mode doesn't count).
  - every input chunk is a separate DRAM tensor, contiguous per
    partition (2-8KB descriptors); sub-slicing one big tensor made
    512-1KB descriptors and dropped input BW from ~310 to ~210 GB/s.
  - inputs stream on both HWDGE rings in first-use order: x8/wq8 (Q),
    x8/wk8 (K), xv/wv (V).
  - SBUF->HBM writes: both HWDGE rings funnel through the same 3 SDMA
    engines (~78 GB/s combined), SWDGE (gpsimd) spreads over ~6 others;
    output rotates sync/scalar/gpsimd, small last block on sync so the
    end-of-kernel drain is short, gpsimd's last block is b=6 (SWDGE
    teardown is slow).
  - PSUM: pproj 2 banks, patt 2, pout 3x1 bank, ptp 1 = 8.
"""

import os
import sys

sys.path.insert(0, "/opt/trn_rl_repo")

import ml_dtypes
import numpy as np

import concourse.bass as bass  # noqa: F401  (bass must import before tile)
import concourse.mybir as mybir
import concourse.tile as tile
from concourse import bacc
from concourse.bass_utils import run_bass_kernel_spmd

F32 = mybir.dt.float32
F16 = mybir.dt.float16
F8 = mybir.dt.float8e4

B, S, H = 4, 2048, 1024
WCTX = 2
NCORES = 8
SHARD = S // 2  # tokens per core
EXT = SHARD + 2 * WCTX  # 1028
EXTP = 1040  # fp8 x tile padded so the hc-axis stride is 16B-aligned
P = 128
QB = 123  # queries per attention block
WIN = QB + 2 * WCTX  # 127 = key window per block
NBLK = (SHARD + QB - 1) // QB  # 9
HC = H // P  # 8 feature chunks
HH = HC // 2
TH = 516  # token split of x between the two projection halves
SCALE = 1.0 / np.sqrt(np.float32(H))
XS, WS = 16.0, 64.0  # fp8 input scales (x, weights)
QKS = XS * WS  # Q/K are computed scaled by 1024

_prog_cache = {}


def _build_program():
    nc = bacc.Bacc("TRN2", target_bir_lowering=False, debug=False)

    def din(name, shape, dt):
        return nc.dram_tensor(name, shape, dt, kind="ExternalInput").ap()

    # fp8 x/wq/wk chunks and fp16 xv/wv chunks, one DRAM tensor per
    # (ring, piece) so every transfer is contiguous per partition
    x8a = [din(f"x8a{i}", [P, HH * TH], F8) for i in range(2)]
    x8b = [din(f"x8b{i}", [P, HH * (EXT - TH)], F8) for i in range(2)]
    # weights quartered (jc-half x hc-half) so the first Q/K matmul group
    # only waits for half the weight bytes
    wq8 = [[din(f"wq8{j}{i}", [P, HH * 512], F8) for i in range(2)] for j in range(2)]
    wk8 = [[din(f"wk8{j}{i}", [P, HH * 512], F8) for i in range(2)] for j in range(2)]
    xv = [din(f"xv{i}", [P, HH * EXT], F16) for i in range(2)]
    wv = [din(f"wv{i}", [P, HH * H], F16) for i in range(2)]
    bq_d = din("bq_c", [P, HC], F32)
    bk_d = din("bk_c", [P, HC], F32)
    bv_d = din("bv_r", [P, H], F16)
    id_d = din("ident", [P, P], F16)
    mk_d = din("mask", [NBLK, QB, WIN], F32)
    out_d = nc.dram_tensor("out", [SHARD, H], F16, kind="ExternalOutput").ap()

    def r3(ap, n):
        return ap.rearrange("p (hc n) -> p hc n", hc=HH)

    mk_r = mk_d.rearrange("b q c -> q b c")

    with tile.TileContext(nc) as tc:
        with (
            tc.tile_pool(name="persist", bufs=1) as pers,
            tc.tile_pool(name="vpool", bufs=4) as vpool,
            tc.tile_pool(name="spool", bufs=4) as spool,
            tc.tile_pool(name="opool", bufs=4) as opool,
            tc.tile_pool(name="pproj", bufs=2, space="PSUM") as pproj,
            tc.tile_pool(name="patt", bufs=2, space="PSUM") as patt,
            tc.tile_pool(name="pout", bufs=3, space="PSUM") as pout,
            tc.tile_pool(name="ptp", bufs=1, space="PSUM") as ptp,
        ):
            # ---- HAM warm-up: real matmuls on a memset tile ----
            warm = pers.tile([P, 640], F16)
            with tc.high_priority():
                nc.vector.memset(warm[:], 0.25)
                for _ in range(10):
                    psw = pproj.tile([P, 512], F32, tag="proj")
                    nc.tensor.matmul(
                        psw[:], warm[:, :128], warm[:, 128:], start=True, stop=True
                    )

            # ---- inputs in first-use order across both HWDGE rings ----
            xt8 = pers.tile([P, HC, EXTP], F8)
            # weights laid out [P, jc-half, hc, 512] so each quarter-DMA is
            # contiguous per partition (2KB descriptors)
            wq8_sb = pers.tile([P, 2, HC, 512], F8)
            wk8_sb = pers.tile([P, 2, HC, 512], F8)
            xtv = pers.tile([P, HC, EXT], F16)
            wv_sb = pers.tile([P, HC, H], F16)
            rings = (nc.sync, nc.scalar)
            for i, ring in enumerate(rings):
                h0, h1 = i * HH, (i + 1) * HH
                ring.dma_start(xt8[:, h0:h1, :TH], r3(x8a[i], TH))
            for j in range(2):
                for i, ring in enumerate(rings):
                    h0, h1 = i * HH, (i + 1) * HH
                    ring.dma_start(wq8_sb[:, j, h0:h1, :], r3(wq8[j][i], 512))
            for i, ring in enumerate(rings):
                h0, h1 = i * HH, (i + 1) * HH
                ring.dma_start(xt8[:, h0:h1, TH:EXT], r3(x8b[i], EXT - TH))
            for j in range(2):
                for i, ring in enumerate(rings):
                    h0, h1 = i * HH, (i + 1) * HH
                    ring.dma_start(wk8_sb[:, j, h0:h1, :], r3(wk8[j][i], 512))
            for i, ring in enumerate(rings):
                h0, h1 = i * HH, (i + 1) * HH
                ring.dma_start(xtv[:, h0:h1, :], r3(xv[i], EXT))
            for i, ring in enumerate(rings):
                h0, h1 = i * HH, (i + 1) * HH
                ring.dma_start(wv_sb[:, h0:h1, :], r3(wv[i], H))

            # ---- small constants on the gpsimd (SWDGE) queue ----
            bqc = pers.tile([P, HC], F32)
            nc.gpsimd.dma_start(bqc[:], bq_d)
            bkc = pers.tile([P, HC], F32)
            nc.gpsimd.dma_start(bkc[:], bk_d)
            ident = pers.tile([P, P], F16)
            nc.gpsimd.dma_start(ident[:], id_d)
            maskt = pers.tile([QB, NBLK, WIN], F32)
            nc.gpsimd.dma_start(maskt[:], mk_r)
            bvr = pers.tile([P, H], F16)
            nc.gpsimd.dma_start(bvr[:], bv_d)

            DR = mybir.MatmulPerfMode.DoubleRow

            # ---- Q^T projection: owned tokens ext [2, 1026) ----
            qt_sb = pers.tile([P, HC, SHARD], F16)
            with nc.named_scope("qproj"):
                for t in range(2):
                    for jc in range(HC):
                        ps = pproj.tile([P, 512], F32, tag="proj")
                        for g in range(HH):
                            nc.tensor.matmul(
                                ps[:],
                                wq8_sb[
                                    :,
                                    jc // 4,
                                    2 * g : 2 * g + 2,
                                    (jc % 4) * P : (jc % 4 + 1) * P,
                                ],
                                xt8[:, 2 * g : 2 * g + 2, 2 + 512 * t : 2 + 512 * (t + 1)],
                                start=(g == 0),
                                stop=(g == HH - 1),
                                perf_mode=DR,
                            )
                        nc.vector.tensor_scalar_add(
                            qt_sb[:, jc, 512 * t : 512 * (t + 1)],
                            ps[:],
                            bqc[:, jc : jc + 1],
                        )

            # ---- K^T projection: all ext tokens [0, 1028); the tiny halo
            # chunk is emitted later, interleaved with the first V block
            # (a burst of 64 N=4 matmuls looks idle to the HAM clock gate
            # and re-throttled the PE right at the attention transition) ----
            kt_sb = pers.tile([P, HC, EXT], F16)

            def emit_k_chunk(t0, t1, jcs):
                n = t1 - t0
                for jc in jcs:
                    ps = pproj.tile([P, 512], F32, tag="proj")
                    for g in range(HH):
                        nc.tensor.matmul(
                            ps[:, :n],
                            wk8_sb[
                                :,
                                jc // 4,
                                2 * g : 2 * g + 2,
                                (jc % 4) * P : (jc % 4 + 1) * P,
                            ],
                            xt8[:, 2 * g : 2 * g + 2, t0:t1],
                            start=(g == 0),
                            stop=(g == HH - 1),
                            perf_mode=DR,
                        )
                    nc.vector.tensor_scalar_add(
                        kt_sb[:, jc, t0:t1], ps[:, :n], bkc[:, jc : jc + 1]
                    )

            with nc.named_scope("kproj"):
                emit_k_chunk(0, 512, range(HC))
                emit_k_chunk(512, 1024, range(HC))

            # ---- attention blocks, software-pipelined by two stages ----
            def blk_geom(b):
                q0 = QB * b
                qb = min(QB, SHARD - q0)
                return q0, qb, qb + 2 * WCTX

            def emit_v_half(b, n, vb):
                q0, qb, w = blk_geom(b)
                psv = pproj.tile([P, 512], F32, tag="proj")
                for hc in range(HC):
                    nc.tensor.matmul(
                        psv[:w, :],
                        xtv[:, hc, q0 : q0 + w],
                        wv_sb[:, hc, 512 * n : 512 * (n + 1)],
                        start=(hc == 0),
                        stop=(hc == HC - 1),
                    )
                nc.scalar.copy(vb[:w, 512 * n : 512 * (n + 1)], psv[:w, :])

            def emit_v_bias(b, vb):
                q0, qb, w = blk_geom(b)
                nc.gpsimd.dma_start(vb[w : w + 1, :], bvr[w : w + 1, :])

            def emit_scores_softmax(b):
                q0, qb, w = blk_geom(b)
                pss = patt.tile([QB, WIN], F32, tag="ps")
                for jc in range(HC):
                    nc.tensor.matmul(
                        pss[:qb, :w],
                        qt_sb[:, jc, q0 : q0 + qb],
                        kt_sb[:, jc, q0 : q0 + w],
                        start=(jc == 0),
                        stop=(jc == HC - 1),
                    )
                sm = spool.tile([QB, WIN], F32, tag="sm")
                nc.vector.tensor_tensor(
                    sm[:qb, :w], pss[:qb, :w], maskt[:qb, b, :w], op=mybir.AluOpType.add
                )
                pexp = spool.tile([QB, WIN], F32, tag="pexp")
                rsum = spool.tile([QB, 1], F32, tag="rsum")
                nc.scalar.activation(
                    pexp[:qb, :w],
                    sm[:qb, :w],
                    mybir.ActivationFunctionType.Exp,
                    bias=0.0,
                    scale=float(SCALE / (QKS * QKS)),
                    accum_out=rsum[:qb],
                )
                rcp = spool.tile([QB, 1], F32, tag="rcp")
                nc.vector.reciprocal(rcp[:qb], rsum[:qb])
                pn = spool.tile([QB, WIN + 1], F16, tag="pn")
                nc.vector.tensor_scalar_mul(pn[:qb, :w], pexp[:qb, :w], rcp[:qb])
                nc.vector.memset(pn[:qb, w : w + 1], 1.0)
                return pn

            def emit_transpose(b, pn):
                q0, qb, w = blk_geom(b)
                pst = ptp.tile([WIN + 1, QB], F16, tag="pt")
                nc.tensor.transpose(pst[: w + 1, :qb], pn[:qb, : w + 1], ident[:qb, :qb])
                pts = spool.tile([WIN + 1, QB], F16, tag="pts")
                nc.scalar.copy(pts[: w + 1, :qb], pst[: w + 1, :qb])
                return pts

            OUTQ = {0: 0, 1: 1, 2: 2, 3: 0, 4: 1, 5: 2, 6: 0, 7: 1, 8: 0}

            def emit_pv_out(b, pts, vb):
                q0, qb, w = blk_geom(b)
                ob = opool.tile([QB, H], F16, tag="ob")
                for n in range(2):
                    pso = pout.tile([QB, 512], F32, tag="po")
                    nc.tensor.matmul(
                        pso[:qb, :],
                        pts[: w + 1, :qb],
                        vb[: w + 1, 512 * n : 512 * (n + 1)],
                        start=True,
                        stop=True,
                    )
                    eng = nc.vector.tensor_copy if n == 0 else nc.scalar.copy
                    eng(ob[:qb, 512 * n : 512 * (n + 1)], pso[:qb, :])
                dma_eng = (nc.sync, nc.scalar, nc.gpsimd)[OUTQ[b]]
                dma_eng.dma_start(out_d[q0 : q0 + qb, :], ob[:qb, :])

            with nc.named_scope("attn"):
                # prologue: blocks 0 and 1, with the tiny K halo chunk
                # sandwiched between dense V matmul bursts
                stage = []
                vb0 = vpool.tile([P, H], F16, tag="vblk")
                emit_v_half(0, 0, vb0)
                emit_k_chunk(1024, EXT, range(0, 4))
                emit_v_half(0, 1, vb0)
                emit_v_bias(0, vb0)
                emit_k_chunk(1024, EXT, range(4, HC))
                stage.append((vb0, emit_scores_softmax(0)))
                vb1 = vpool.tile([P, H], F16, tag="vblk")
                emit_v_half(1, 0, vb1)
                emit_v_half(1, 1, vb1)
                emit_v_bias(1, vb1)
                stage.append((vb1, emit_scores_softmax(1)))
                for b in range(NBLK):
                    vb, pn = stage[b]
                    vb2 = None
                    if b + 2 < NBLK:
                        vb2 = vpool.tile([P, H], F16, tag="vblk")
                        emit_v_half(b + 2, 0, vb2)
                    pts = emit_transpose(b, pn)
                    if vb2 is not None:
                        emit_v_half(b + 2, 1, vb2)
                        emit_v_bias(b + 2, vb2)
                    emit_pv_out(b, pts, vb)
                    if vb2 is not None:
                        stage.append((vb2, emit_scores_softmax(b + 2)))

    nc.compile()
    return nc


def _build_mask(h: int) -> np.ndarray:
    mask = np.full((NBLK, QB, WIN), -1e30, dtype=np.float32)
    r = np.arange(QB)[:, None]
    c = np.arange(WIN)[None, :]
    band = (c - r >= 0) & (c - r <= 2 * WCTX)
    for b in range(NBLK):
        q0 = QB * b
        qb = min(QB, SHARD - q0)
        gk = h * SHARD + q0 + c - WCTX  # global key token index
        valid = band & (gk >= 0) & (gk < S) & (r < qb) & (c < qb + 2 * WCTX)
        mask[b] = np.where(valid, np.float32(0.0), np.float32(-1e30))
    return mask


def _pmaj(w: np.ndarray, dtype) -> list[np.ndarray]:
    """[H, N] -> two ring-halves [P, HH*N], partition-contiguous."""
    n = w.shape[1]
    a = w.reshape(HC, P, n).transpose(1, 0, 2)  # [P, HC, n]
    return [
        np.ascontiguousarray(a[:, i * HH : (i + 1) * HH].reshape(P, HH * n)).astype(
            dtype
        )
        for i in range(2)
    ]


def kernel(sequence_output, Wq, bq, Wk, bk, Wv, bv):
    x = np.asarray(sequence_output, dtype=np.float32)
    Wq = np.asarray(Wq, dtype=np.float32)
    Wk = np.asarray(Wk, dtype=np.float32)
    Wv = np.asarray(Wv, dtype=np.float32)
    bq = np.asarray(bq, dtype=np.float32)
    bk = np.asarray(bk, dtype=np.float32)
    bv = np.asarray(bv, dtype=np.float32)

    if "nc" not in _prog_cache:
        _prog_cache["nc"] = _build_program()
    nc = _prog_cache["nc"]

    f8 = ml_dtypes.float8_e4m3fn
    wq8 = [_pmaj(Wq[:, 512 * j : 512 * (j + 1)] * WS, f8) for j in range(2)]
    wk8 = [_pmaj(Wk[:, 512 * j : 512 * (j + 1)] * WS, f8) for j in range(2)]
    wv_h = _pmaj(Wv, np.float16)
    bq_c = np.ascontiguousarray(bq.reshape(HC, P).T) * np.float32(QKS)
    bk_c = np.ascontiguousarray(bk.reshape(HC, P).T) * np.float32(QKS)
    bv_r = np.ascontiguousarray(np.broadcast_to(bv, (P, H))).astype(np.float16)
    ident = np.eye(P, dtype=np.float16)
    masks = [_build_mask(0), _build_mask(1)]

    # pad each sequence with WCTX zero rows on both ends, slice ext windows
    xp = np.zeros((B, S + 2 * WCTX, H), dtype=np.float32)
    xp[:, WCTX : WCTX + S] = x

    in_maps = []
    for c in range(NCORES):
        bidx, h = divmod(c, 2)
        ext = xp[bidx, h * SHARD : h * SHARD + EXT]  # [EXT, H]
        xT = np.ascontiguousarray(ext.T)  # [H, EXT]
        x8a = _pmaj(xT[:, :TH] * XS, f8)
        x8b = _pmaj(xT[:, TH:] * XS, f8)
        xv = _pmaj(xT, np.float16)
        im = {
            "bq_c": bq_c,
            "bk_c": bk_c,
            "bv_r": bv_r,
            "ident": ident,
            "mask": masks[h],
        }
        for i in range(2):
            im[f"x8a{i}"] = x8a[i]
            im[f"x8b{i}"] = x8b[i]
            im[f"xv{i}"] = xv[i]
            im[f"wv{i}"] = wv_h[i]
            for j in range(2):
                im[f"wq8{j}{i}"] = wq8[j][i]
                im[f"wk8{j}{i}"] = wk8[j][i]
        in_maps.append(im)

    trace = bool(int(os.environ.get("LK_TRACE", "0")))
    res = run_bass_kernel_spmd(
        nc,
        in_maps,
        core_ids=list(range(NCORES)),
        trace=trace,
        trace_cores=list(range(NCORES)) if trace else None,
    )
    _prog_cache["last_results"] = res

    out = np.empty((B, S, H), dtype=np.float32)
    for c in range(NCORES):
        bidx, h = divmod(c, 2)
        out[bidx, h * SHARD : (h + 1) * SHARD] = res.results[c]["out"].astype(
            np.float32
        )
    return out


# revision 25
# speedup vs baseline: 1.1533x; 1.0226x over previous
"""Local (sliding-window w=2) attention, B=4 S=2048 H=1024, on 8 trn2 cores.

Strategy: sequence-parallel. Each core owns half of one batch's sequence
(1024 tokens) plus a 2-token halo on each side (ext = 1028 tokens).

Per core:
  Q^T/K^T projections feature-major [H, T] in *fp8e4m3 with
  perf_mode=DoubleRow* (K=256 per matmul, ~1.7x the fp16 rate; x scaled
  by 16 and W by 64 into the fp8 sweet range, biases pre-scaled by 1024
  on host, the 1/1024^2 descale folded into the softmax exp scale;
  measured end-to-end rel err 1.55e-2 vs the 2e-2 gate).  V stays fp16
  (its quantization error would land directly in the output).  Then 9
  q-blocks of 123 queries: band scores (window 127) off the scaled fp16
  Q^T/K^T, masked softmax (ACT exp + fused row-sum), P transpose on PE,
  P@V (bias via extra V row + ones column), fp16 output.

Lessons baked in from traces of previous versions:
  - HAM warm-up needs *real* matmuls (transpose-mode doesn't count).
  - every input chunk is a separate DRAM tensor, contiguous per
    partition (2-8KB descriptors); sub-slicing one big tensor made
    512-1KB descriptors and dropped input BW from ~310 to ~210 GB/s.
  - inputs stream on both HWDGE rings in first-use order: x8/wq8 (Q),
    x8/wk8 (K), xv/wv (V).
  - SBUF->HBM writes: both HWDGE rings funnel through the same 3 SDMA
    engines (~78 GB/s combined), SWDGE (gpsimd) spreads over ~6 others;
    output rotates sync/scalar/gpsimd, small last block on sync so the
    end-of-kernel drain is short, gpsimd's last block is b=6 (SWDGE
    teardown is slow).
  - PSUM: pproj 2 banks, patt 2, pout 3x1 bank, ptp 1 = 8.
"""

import os
import sys

sys.path.insert(0, "/opt/trn_rl_repo")

import ml_dtypes
import numpy as np

import concourse.bass as bass  # noqa: F401  (bass must import before tile)
import concourse.mybir as mybir
import concourse.tile as tile
from concourse import bacc
from concourse.bass_utils import run_bass_kernel_spmd

F32 = mybir.dt.float32
F16 = mybir.dt.float16
F8 = mybir.dt.float8e4

B, S, H = 4, 2048, 1024
WCTX = 2
NCORES = 8
SHARD = S // 2  # tokens per core
EXT = SHARD + 2 * WCTX  # 1028
EXTP = 1040  # fp8 x tile padded so the hc-axis stride is 16B-aligned
P = 128
QB = 123  # queries per attention block
WIN = QB + 2 * WCTX  # 127 = key window per block
NBLK = (SHARD + QB - 1) // QB  # 9
HC = H // P  # 8 feature chunks
HH = HC // 2
TH = 516  # token split of x between the two projection halves
SCALE = 1.0 / np.sqrt(np.float32(H))
XS, WS = 16.0, 64.0  # fp8 input scales (x, weights)
QKS = XS * WS  # Q/K are computed scaled by 1024

_prog_cache = {}


def _build_program():
    nc = bacc.Bacc("TRN2", target_bir_lowering=False, debug=False)

    def din(name, shape, dt):
        return nc.dram_tensor(name, shape, dt, kind="ExternalInput").ap()

    # fp8 x/wq/wk chunks and fp16 xv/wv chunks, one DRAM tensor per
    # (ring, piece) so every transfer is contiguous per partition
    x8a = [din(f"x8a{i}", [P, HH * TH], F8) for i in range(2)]
    x8b = [din(f"x8b{i}", [P, HH * (EXT - TH)], F8) for i in range(2)]
    # weights quartered (jc-half x hc-half) so the first Q/K matmul group
    # only waits for half the weight bytes
    wq8 = [[din(f"wq8{j}{i}", [P, HH * 512], F8) for i in range(2)] for j in range(2)]
    wk8 = [[din(f"wk8{j}{i}", [P, HH * 512], F8) for i in range(2)] for j in range(2)]
    xv = [din(f"xv{i}", [P, HH * EXT], F16) for i in range(2)]
    wv = [din(f"wv{i}", [P, HH * H], F16) for i in range(2)]
    bq_d = din("bq_c", [P, HC], F32)
    bk_d = din("bk_c", [P, HC], F32)
    bv_d = din("bv_r", [P, H], F16)
    id_d = din("ident", [P, P], F16)
    mk_d = din("mask", [NBLK, QB, WIN], F32)
    out_d = nc.dram_tensor("out", [SHARD, H], F16, kind="ExternalOutput").ap()

    def r3(ap, n):
        return ap.rearrange("p (hc n) -> p hc n", hc=HH)

    mk_r = mk_d.rearrange("b q c -> q b c")

    with tile.TileContext(nc) as tc:
        with (
            tc.tile_pool(name="persist", bufs=1) as pers,
            tc.tile_pool(name="vpool", bufs=4) as vpool,
            tc.tile_pool(name="spool", bufs=4) as spool,
            tc.tile_pool(name="opool", bufs=4) as opool,
            tc.tile_pool(name="pproj", bufs=2, space="PSUM") as pproj,
            tc.tile_pool(name="patt", bufs=2, space="PSUM") as patt,
            tc.tile_pool(name="pout", bufs=3, space="PSUM") as pout,
            tc.tile_pool(name="ptp", bufs=1, space="PSUM") as ptp,
        ):
            # ---- HAM warm-up: real matmuls on a memset tile ----
            warm = pers.tile([P, 640], F16)
            with tc.high_priority():
                nc.vector.memset(warm[:], 0.25)
                for _ in range(10):
                    psw = pproj.tile([P, 512], F32, tag="proj")
                    nc.tensor.matmul(
                        psw[:], warm[:, :128], warm[:, 128:], start=True, stop=True
                    )

            # ---- inputs in first-use order across both HWDGE rings ----
            xt8 = pers.tile([P, HC, EXTP], F8)
            # weights laid out [P, jc-half, hc, 512] so each quarter-DMA is
            # contiguous per partition (2KB descriptors)
            wq8_sb = pers.tile([P, 2, HC, 512], F8)
            wk8_sb = pers.tile([P, 2, HC, 512], F8)
            xtv = pers.tile([P, HC, EXT], F16)
            wv_sb = pers.tile([P, HC, H], F16)
            rings = (nc.sync, nc.scalar)
            for i, ring in enumerate(rings):
                h0, h1 = i * HH, (i + 1) * HH
                ring.dma_start(xt8[:, h0:h1, :TH], r3(x8a[i], TH))
            for j in range(2):
                for i, ring in enumerate(rings):
                    h0, h1 = i * HH, (i + 1) * HH
                    ring.dma_start(wq8_sb[:, j, h0:h1, :], r3(wq8[j][i], 512))
            for i, ring in enumerate(rings):
                h0, h1 = i * HH, (i + 1) * HH
                ring.dma_start(xt8[:, h0:h1, TH:EXT], r3(x8b[i], EXT - TH))
            for j in range(2):
                for i, ring in enumerate(rings):
                    h0, h1 = i * HH, (i + 1) * HH
                    ring.dma_start(wk8_sb[:, j, h0:h1, :], r3(wk8[j][i], 512))
            for i, ring in enumerate(rings):
                h0, h1 = i * HH, (i + 1) * HH
                ring.dma_start(xtv[:, h0:h1, :], r3(xv[i], EXT))
            for i, ring in enumerate(rings):
                h0, h1 = i * HH, (i + 1) * HH
                ring.dma_start(wv_sb[:, h0:h1, :], r3(wv[i], H))

            # ---- small constants on the gpsimd (SWDGE) queue ----
            bqc = pers.tile([P, HC], F32)
            nc.gpsimd.dma_start(bqc[:], bq_d)
            bkc = pers.tile([P, HC], F32)
            nc.gpsimd.dma_start(bkc[:], bk_d)
            ident = pers.tile([P, P], F16)
            nc.gpsimd.dma_start(ident[:], id_d)
            maskt = pers.tile([QB, NBLK, WIN], F32)
            nc.gpsimd.dma_start(maskt[:], mk_r)
            bvr = pers.tile([P, H], F16)
            nc.gpsimd.dma_start(bvr[:], bv_d)

            DR = mybir.MatmulPerfMode.DoubleRow

            # ---- Q^T projection: owned tokens ext [2, 1026) ----
            qt_sb = pers.tile([P, HC, SHARD], F16)
            with nc.named_scope("qproj"):
                for t in range(2):
                    for jc in range(HC):
                        ps = pproj.tile([P, 512], F32, tag="proj")
                        for g in range(HH):
                            nc.tensor.matmul(
                                ps[:],
                                wq8_sb[
                                    :,
                                    jc // 4,
                                    2 * g : 2 * g + 2,
                                    (jc % 4) * P : (jc % 4 + 1) * P,
                                ],
                                xt8[:, 2 * g : 2 * g + 2, 2 + 512 * t : 2 + 512 * (t + 1)],
                                start=(g == 0),
                                stop=(g == HH - 1),
                                perf_mode=DR,
                            )
                        nc.vector.tensor_scalar_add(
                            qt_sb[:, jc, 512 * t : 512 * (t + 1)],
                            ps[:],
                            bqc[:, jc : jc + 1],
                        )

            # ---- K^T projection: all ext tokens [0, 1028); the tiny halo
            # chunk is emitted later, interleaved with the first V block
            # (a burst of 64 N=4 matmuls looks idle to the HAM clock gate
            # and re-throttled the PE right at the attention transition) ----
            kt_sb = pers.tile([P, HC, EXT], F16)

            def emit_k_chunk(t0, t1, jcs):
                n = t1 - t0
                for jc in jcs:
                    ps = pproj.tile([P, 512], F32, tag="proj")
                    for g in range(HH):
                        nc.tensor.matmul(
                            ps[:, :n],
                            wk8_sb[
                                :,
                                jc // 4,
                                2 * g : 2 * g + 2,
                                (jc % 4) * P : (jc % 4 + 1) * P,
                            ],
                            xt8[:, 2 * g : 2 * g + 2, t0:t1],
                            start=(g == 0),
                            stop=(g == HH - 1),
                            perf_mode=DR,
                        )
                    nc.vector.tensor_scalar_add(
                        kt_sb[:, jc, t0:t1], ps[:, :n], bkc[:, jc : jc + 1]
                    )

            with nc.named_scope("kproj"):
                emit_k_chunk(0, 512, range(HC))
                emit_k_chunk(512, 1024, range(HC))

            # ---- attention blocks, software-pipelined by two stages ----
            def blk_geom(b):
                q0 = QB * b
                qb = min(QB, SHARD - q0)
                return q0, qb, qb + 2 * WCTX

            def emit_v_half(b, n, vb):
                q0, qb, w = blk_geom(b)
                psv = pproj.tile([P, 512], F32, tag="proj")
                for hc in range(HC):
                    nc.tensor.matmul(
                        psv[:w, :],
                        xtv[:, hc, q0 : q0 + w],
                        wv_sb[:, hc, 512 * n : 512 * (n + 1)],
                        start=(hc == 0),
                        stop=(hc == HC - 1),
                    )
                nc.scalar.copy(vb[:w, 512 * n : 512 * (n + 1)], psv[:w, :])

            def emit_v_bias(b, vb):
                q0, qb, w = blk_geom(b)
                nc.gpsimd.dma_start(vb[w : w + 1, :], bvr[w : w + 1, :])

            def emit_scores_softmax(b):
                q0, qb, w = blk_geom(b)
                pss = patt.tile([QB, WIN], F32, tag="ps")
                for jc in range(HC):
                    nc.tensor.matmul(
                        pss[:qb, :w],
                        qt_sb[:, jc, q0 : q0 + qb],
                        kt_sb[:, jc, q0 : q0 + w],
                        start=(jc == 0),
                        stop=(jc == HC - 1),
                    )
                sm = spool.tile([QB, WIN], F32, tag="sm")
                nc.vector.tensor_tensor(
                    sm[:qb, :w], pss[:qb, :w], maskt[:qb, b, :w], op=mybir.AluOpType.add
                )
                pexp = spool.tile([QB, WIN], F32, tag="pexp")
                rsum = spool.tile([QB, 1], F32, tag="rsum")
                nc.scalar.activation(
                    pexp[:qb, :w],
                    sm[:qb, :w],
                    mybir.ActivationFunctionType.Exp,
                    bias=0.0,
                    scale=float(SCALE / (QKS * QKS)),
                    accum_out=rsum[:qb],
                )
                rcp = spool.tile([QB, 1], F32, tag="rcp")
                nc.vector.reciprocal(rcp[:qb], rsum[:qb])
                pn = spool.tile([QB, WIN + 1], F16, tag="pn")
                nc.vector.tensor_scalar_mul(pn[:qb, :w], pexp[:qb, :w], rcp[:qb])
                nc.vector.memset(pn[:qb, w : w + 1], 1.0)
                return pn

            def emit_transpose(b, pn):
                q0, qb, w = blk_geom(b)
                pst = ptp.tile([WIN + 1, QB], F16, tag="pt")
                nc.tensor.transpose(pst[: w + 1, :qb], pn[:qb, : w + 1], ident[:qb, :qb])
                pts = spool.tile([WIN + 1, QB], F16, tag="pts")
                nc.scalar.copy(pts[: w + 1, :qb], pst[: w + 1, :qb])
                return pts

            OUTQ = {0: 0, 1: 1, 2: 2, 3: 0, 4: 1, 5: 2, 6: 0, 7: 1, 8: 0}

            def emit_pv_out(b, pts, vb):
                q0, qb, w = blk_geom(b)
                ob = opool.tile([QB, H], F16, tag="ob")
                for n in range(2):
                    pso = pout.tile([QB, 512], F32, tag="po")
                    nc.tensor.matmul(
                        pso[:qb, :],
                        pts[: w + 1, :qb],
                        vb[: w + 1, 512 * n : 512 * (n + 1)],
                        start=True,
                        stop=True,
                    )
                    eng = nc.vector.tensor_copy if n == 0 else nc.scalar.copy
                    eng(ob[:qb, 512 * n : 512 * (n + 1)], pso[:qb, :])
                dma_eng = (nc.sync, nc.scalar, nc.gpsimd)[OUTQ[b]]
                dma_eng.dma_start(out_d[q0 : q0 + qb, :], ob[:qb, :])

            with nc.named_scope("attn"):
                # prologue: blocks 0 and 1, with the tiny K halo chunk
                # sandwiched between dense V matmul bursts
                stage = []
                vb0 = vpool.tile([P, H], F16, tag="vblk")
                emit_v_half(0, 0, vb0)
                emit_k_chunk(1024, EXT, range(0, 4))
                emit_v_half(0, 1, vb0)
                emit_v_bias(0, vb0)
                emit_k_chunk(1024, EXT, range(4, HC))
                stage.append((vb0, emit_scores_softmax(0)))
                vb1 = vpool.tile([P, H], F16, tag="vblk")
                emit_v_half(1, 0, vb1)
                emit_v_half(1, 1, vb1)
                emit_v_bias(1, vb1)
                stage.append((vb1, emit_scores_softmax(1)))
                # transpose+pts first: the pts copy heads the ACT queue and is
                # long done when PV issues after the V burst
                for b in range(NBLK):
                    vb, pn = stage[b]
                    pts = emit_transpose(b, pn)
                    vb2 = None
                    if b + 2 < NBLK:
                        vb2 = vpool.tile([P, H], F16, tag="vblk")
                        emit_v_half(b + 2, 0, vb2)
                        emit_v_half(b + 2, 1, vb2)
                        emit_v_bias(b + 2, vb2)
                    emit_pv_out(b, pts, vb)
                    if vb2 is not None:
                        stage.append((vb2, emit_scores_softmax(b + 2)))

    nc.compile()
    return nc


def _build_mask(h: int) -> np.ndarray:
    mask = np.full((NBLK, QB, WIN), -1e30, dtype=np.float32)
    r = np.arange(QB)[:, None]
    c = np.arange(WIN)[None, :]
    band = (c - r >= 0) & (c - r <= 2 * WCTX)
    for b in range(NBLK):
        q0 = QB * b
        qb = min(QB, SHARD - q0)
        gk = h * SHARD + q0 + c - WCTX  # global key token index
        valid = band & (gk >= 0) & (gk < S) & (r < qb) & (c < qb + 2 * WCTX)
        mask[b] = np.where(valid, np.float32(0.0), np.float32(-1e30))
    return mask


def _pmaj(w: np.ndarray, dtype) -> list[np.ndarray]:
    """[H, N] -> two ring-halves [P, HH*N], partition-contiguous."""
    n = w.shape[1]
    a = w.reshape(HC, P, n).transpose(1, 0, 2)  # [P, HC, n]
    return [
        np.ascontiguousarray(a[:, i * HH : (i + 1) * HH].reshape(P, HH * n)).astype(
            dtype
        )
        for i in range(2)
    ]


def kernel(sequence_output, Wq, bq, Wk, bk, Wv, bv):
    x = np.asarray(sequence_output, dtype=np.float32)
    Wq = np.asarray(Wq, dtype=np.float32)
    Wk = np.asarray(Wk, dtype=np.float32)
    Wv = np.asarray(Wv, dtype=np.float32)
    bq = np.asarray(bq, dtype=np.float32)
    bk = np.asarray(bk, dtype=np.float32)
    bv = np.asarray(bv, dtype=np.float32)

    if "nc" not in _prog_cache:
        _prog_cache["nc"] = _build_program()
    nc = _prog_cache["nc"]

    f8 = ml_dtypes.float8_e4m3fn
    wq8 = [_pmaj(Wq[:, 512 * j : 512 * (j + 1)] * WS, f8) for j in range(2)]
    wk8 = [_pmaj(Wk[:, 512 * j : 512 * (j + 1)] * WS, f8) for j in range(2)]
    wv_h = _pmaj(Wv, np.float16)
    bq_c = np.ascontiguousarray(bq.reshape(HC, P).T) * np.float32(QKS)
    bk_c = np.ascontiguousarray(bk.reshape(HC, P).T) * np.float32(QKS)
    bv_r = np.ascontiguousarray(np.broadcast_to(bv, (P, H))).astype(np.float16)
    ident = np.eye(P, dtype=np.float16)
    masks = [_build_mask(0), _build_mask(1)]

    # pad each sequence with WCTX zero rows on both ends, slice ext windows
    xp = np.zeros((B, S + 2 * WCTX, H), dtype=np.float32)
    xp[:, WCTX : WCTX + S] = x

    in_maps = []
    for c in range(NCORES):
        bidx, h = divmod(c, 2)
        ext = xp[bidx, h * SHARD : h * SHARD + EXT]  # [EXT, H]
        xT = np.ascontiguousarray(ext.T)  # [H, EXT]
        x8a = _pmaj(xT[:, :TH] * XS, f8)
        x8b = _pmaj(xT[:, TH:] * XS, f8)
        xv = _pmaj(xT, np.float16)
        im = {
            "bq_c": bq_c,
            "bk_c": bk_c,
            "bv_r": bv_r,
            "ident": ident,
            "mask": masks[h],
        }
        for i in range(2):
            im[f"x8a{i}"] = x8a[i]
            im[f"x8b{i}"] = x8b[i]
            im[f"xv{i}"] = xv[i]
            im[f"wv{i}"] = wv_h[i]
            for j in range(2):
                im[f"wq8{j}{i}"] = wq8[j][i]
                im[f"wk8{j}{i}"] = wk8[j][i]
        in_maps.append(im)

    trace = bool(int(os.environ.get("LK_TRACE", "0")))
    res = run_bass_kernel_spmd(
        nc,
        in_maps,
        core_ids=list(range(NCORES)),
        trace=trace,
        trace_cores=list(range(NCORES)) if trace else None,
    )
    _prog_cache["last_results"] = res

    out = np.empty((B, S, H), dtype=np.float32)
    for c in range(NCORES):
        bidx, h = divmod(c, 2)
        out[bidx, h * SHARD : (h + 1) * SHARD] = res.results[c]["out"].astype(
            np.float32
        )
    return out
